# revision 17
# baseline (speedup 1.0000x reference)
"""KMeans inference (argmin over squared distances) on 8 Trainium2 cores.

Problem: features [262144, 768] fp32, cluster_centers [1024, 768] fp32.
Output: argmin_k ||x_i - c_k||^2 as int32 [262144].

Strategy (data-parallel over rows, fp8 DoubleRow matmul):
  - argmin_k ||x-c_k||^2 == argmax_k (x.c_k - 0.5*||c_k||^2); the ||x||^2
    term is constant per row and drops out of the argmin.
  - Shard rows across 8 cores (32768 rows/core). Host pre-casts to fp8
    (e4m3, max-240 TRN variant) and pre-transposes each shard to
    xT [768, 32768]. fp8 DoubleRow runs the PE at 2 MACs/cell/cycle,
    halving tensor-engine time vs bf16/fp32r.
  - Dims 765..767 are sacrificed to carry the bias inside the matmul:
    x rows 765..767 = (4, 1, 1); centroid rows 765..767 = a 3-term fp8
    decomposition of -0.5*||c8||^2. Scores land in PSUM already biased, so
    the scalar/vector/gpsimd bias pipeline disappears entirely.
  - ACT casts PSUM fp32 -> SBUF fp16; DVE folds the 1024 scores twice by
    pairwise max (1024 -> 512 -> 256), then MAX8 + FIND_INDEX8 scan only
    256 elements, writing top-8 fold-values/positions straight into
    staging tiles. A fold position i means "centroid i, i+256, i+512 or
    i+768" - the host disambiguates by exact re-scoring.
  - Host: every row exactly re-scores its candidates (4 for confident
    rows, 32 from the top-8 fold positions for rows whose folded top-2
    gap is under GAP_THRESHOLD, ~77%). Validated to leave ~0 mismatches
    vs the fp32 reference (fp8 noise std ~2.7).
"""

import sys

sys.path.insert(0, "/opt/trn_rl_repo")

import numpy as np

N_CORES = 8
N, K, D = 262144, 1024, 768
D_KEEP = 765                           # data dims kept; 765..767 carry bias
ROWS_PER_CORE = N // N_CORES          # 32768
SLAB_ROWS = 1024                       # steady-state rows per DMA slab
N_ROWTILES = ROWS_PER_CORE // 128      # 256
D_TILES = D // 128                     # 6
D_PAIRS = D_TILES // 2                 # 3 DoubleRow pairs of 256

# Device score error (fp8 quantization of x and c + 3 dropped dims + 3-term
# fp8 bias + fp16 score cast) has std ~2.7. Rows with folded top-2 gap <
# GAP_THRESHOLD get a 16-candidate exact host re-score (top-8 fold
# positions x 2 halves); confident rows a 2-candidate one.
GAP_THRESHOLD = 15.0

_PROGRAM = None


def _build_program():
    import concourse.mybir as mybir
    from concourse import bacc
    from concourse.tile import TileContext

    F32 = mybir.dt.float32
    F16 = mybir.dt.float16
    F8 = mybir.dt.float8e4
    U32 = mybir.dt.uint32
    DR = mybir.MatmulPerfMode.DoubleRow

    nc = bacc.Bacc()
    xt = nc.declare_dram_parameter("xt", [D, ROWS_PER_CORE], F8, isOutput=False)
    cbt = nc.declare_dram_parameter("cbt", [D, K], F8, isOutput=False)
    # Outputs: idx8[p, 8m:8m+8] / val8[p, 8m:8m+8] = top-8 indices / biased
    # score values of row m*128 + p, descending.
    out_idx = nc.declare_dram_parameter(
        "idx8", [128, 8 * N_ROWTILES], U32, isOutput=True
    )
    out_val = nc.declare_dram_parameter(
        "val8", [128, 8 * N_ROWTILES], F16, isOutput=True
    )

    with TileContext(nc) as tc:
        with (
            tc.tile_pool(name="consts", bufs=1) as consts,
            tc.tile_pool(name="xslab", bufs=4) as xslab_pool,
            tc.tile_pool(name="scores", bufs=6) as scores_pool,
            tc.tile_pool(name="stage", bufs=3) as stage_pool,
            tc.tile_pool(name="psum", bufs=4, space="PSUM") as psum_pool,
        ):
            xt_v = xt.rearrange("(t p) r -> p t r", p=128)

            # Warm-up slabs keep the first matmul gate small.
            slab_rows = [256, 256, 512] + [SLAB_ROWS] * (
                (ROWS_PER_CORE - 1024) // SLAB_ROWS
            )
            assert sum(slab_rows) == ROWS_PER_CORE

            xs0 = xslab_pool.tile([128, D_TILES, slab_rows[0]], F8, tag="xs")
            for dt in range(D_TILES):
                nc.sync.dma_start(
                    out=xs0[:, dt : dt + 1, :],
                    in_=xt_v[:, dt : dt + 1, 0 : slab_rows[0]],
                )
            cb = consts.tile([128, D_TILES, K], F8, tag="cb")
            for dt in range(D_TILES):
                nc.sync.dma_start(
                    out=cb[:, dt : dt + 1, :],
                    in_=cbt.rearrange("(t p) k -> p t k", p=128)[:, dt : dt + 1, :],
                )

            CHUNK_RT = 32  # row-tiles per output staging chunk
            stage_idx = stage_val = None
            mc = 0  # global row-tile counter
            r0 = 0
            for slab, rows in enumerate(slab_rows):
                if slab == 0:
                    xs = xs0
                else:
                    xs = xslab_pool.tile([128, D_TILES, rows], F8, tag="xs")
                    nc.sync.dma_start(out=xs, in_=xt_v[:, :, r0 : r0 + rows])
                for sub in range(rows // 128):
                    if mc % CHUNK_RT == 0:
                        stage_idx = stage_pool.tile(
                            [128, 8 * CHUNK_RT], U32, tag="sidx"
                        )
                        stage_val = stage_pool.tile(
                            [128, 8 * CHUNK_RT], F16, tag="sval"
                        )
                    mo = mc % CHUNK_RT
                    ps = psum_pool.tile([128, K], F32, tag="ps")
                    for pi in range(D_PAIRS):
                        dt = 2 * pi
                        xst = xs[:, dt : dt + 2, sub * 128 : (sub + 1) * 128]
                        nc.tensor.matmul(
                            ps[:, 0:512],
                            xst,
                            cb[:, dt : dt + 2, 0:512],
                            start=(pi == 0),
                            stop=(pi == D_PAIRS - 1),
                            perf_mode=DR,
                        )
                        nc.tensor.matmul(
                            ps[:, 512:1024],
                            xst,
                            cb[:, dt : dt + 2, 512:1024],
                            start=(pi == 0),
                            stop=(pi == D_PAIRS - 1),
                            perf_mode=DR,
                        )
                    s16 = scores_pool.tile([128, K], F16, tag="s16")
                    nc.scalar.copy(s16, ps)
                    fold = scores_pool.tile([128, K // 2], F16, tag="fold")
                    nc.vector.tensor_max(fold, s16[:, 0:512], s16[:, 512:1024])
                    fold2 = scores_pool.tile([128, K // 4], F16, tag="fold2")
                    nc.vector.tensor_max(fold2, fold[:, 0:256], fold[:, 256:512])
                    v8 = stage_val[:, 8 * mo : 8 * mo + 8]
                    i8 = stage_idx[:, 8 * mo : 8 * mo + 8]
                    nc.vector.max(out=v8, in_=fold2)
                    nc.vector.max_index(out=i8, in_max=v8, in_values=fold2)
                    if mc % CHUNK_RT == CHUNK_RT - 1:
                        m0 = mc - (CHUNK_RT - 1)
                        nc.sync.dma_start(
                            out=out_idx[:, 8 * m0 : 8 * (m0 + CHUNK_RT)],
                            in_=stage_idx,
                        )
                        nc.sync.dma_start(
                            out=out_val[:, 8 * m0 : 8 * (m0 + CHUNK_RT)],
                            in_=stage_val,
                        )
                    mc += 1
                r0 += rows

    nc.finalize()
    return nc


def _get_program():
    global _PROGRAM
    if _PROGRAM is None:
        _PROGRAM = _build_program()
    return _PROGRAM


def _f8(a):
    import ml_dtypes

    return a.astype(ml_dtypes.float8_e4m3)


def _make_in_maps(features, cluster_centers):
    import ml_dtypes

    E4 = ml_dtypes.float8_e4m3
    c8 = _f8(cluster_centers[:, :D_KEEP])                      # [K, 765] fp8
    b = -0.5 * (c8.astype(np.float64) ** 2).sum(axis=1)        # exact fp8 norms
    t1 = (b / 4).astype(E4)
    t2 = (b - 4 * t1.astype(np.float64)).astype(E4)
    t3 = (b - 4 * t1.astype(np.float64) - t2.astype(np.float64)).astype(E4)
    cbt = np.empty((D, K), dtype=E4)
    cbt[:D_KEEP] = c8.T
    cbt[765] = t1
    cbt[766] = t2
    cbt[767] = t3

    xq = np.empty((N, D), dtype=E4)
    xq[:, :D_KEEP] = _f8(features[:, :D_KEEP])
    xq[:, 765] = E4(4.0)
    xq[:, 766] = E4(1.0)
    xq[:, 767] = E4(1.0)
    xqt = xq.T  # [768, N] view
    in_maps = []
    for i in range(N_CORES):
        xtr = np.ascontiguousarray(
            xqt[:, i * ROWS_PER_CORE : (i + 1) * ROWS_PER_CORE]
        )
        in_maps.append({"xt": xtr, "cbt": cbt})
    return in_maps


def _postprocess(res, features, cluster_centers):
    """Disambiguate fold positions by exact re-scoring of candidates."""
    idx_parts = []
    gap_parts = []
    for i in range(N_CORES):
        idx8 = res.results[i]["idx8"].reshape(128, N_ROWTILES, 8)
        val8 = res.results[i]["val8"].reshape(128, N_ROWTILES, 8)
        # row r = m*128 + p
        idx_parts.append(idx8.transpose(1, 0, 2).reshape(-1, 8))
        v = val8.astype(np.float32)
        gap_parts.append((v[:, :, 0] - v[:, :, 1]).T.reshape(-1))
    top8 = np.concatenate(idx_parts).astype(np.int64)  # [N, 8] fold positions
    gap = np.concatenate(gap_parts)
    out = np.empty(N, dtype=np.int32)

    c = cluster_centers.astype(np.float32)
    cb = -0.5 * (c.astype(np.float64) ** 2).sum(axis=1).astype(np.float32)
    risky = gap < GAP_THRESHOLD
    offs = np.array([0, 256, 512, 768], dtype=np.int64)
    CH = 16384
    for sel, nbase in ((~risky, 1), (risky, 8)):
        rows = np.flatnonzero(sel)
        for s in range(0, rows.size, CH):
            rr = rows[s : s + CH]
            base = top8[rr, :nbase]               # [n, nbase] fold2 positions
            cand = (base[:, :, None] + offs[None, None, :]).reshape(rr.size, -1)
            xr = features[rr].astype(np.float32)  # [n, 768]
            cc = c[cand]                          # [n, 4*nbase, 768]
            sc = np.einsum("nd,nkd->nk", xr, cc, optimize=True) + cb[cand]
            out[rr] = cand[np.arange(rr.size), sc.argmax(axis=1)].astype(np.int32)
    return out


def kernel(features: np.ndarray, cluster_centers: np.ndarray) -> np.ndarray:
    from concourse.bass_utils import run_bass_kernel_spmd

    features = np.ascontiguousarray(features, dtype=np.float32)
    cluster_centers = np.ascontiguousarray(cluster_centers, dtype=np.float32)

    in_maps = _make_in_maps(features, cluster_centers)
    nc = _get_program()
    res = run_bass_kernel_spmd(nc, in_maps, core_ids=list(range(N_CORES)))
    return _postprocess(res, features, cluster_centers)


if __name__ == "__main__":
    rng = np.random.default_rng(0)
    f = rng.standard_normal((N, D)).astype(np.float32)
    c = rng.standard_normal((K, D)).astype(np.float32)
    got = kernel(f, c)
    d2 = (
        (f**2).sum(1, keepdims=True)
        - 2.0 * f @ c.T
        + (c**2).sum(1)
    )
    want = d2.argmin(1)
    print("mismatches:", (got != want).sum(), "/", N)
